# revision 54
# baseline (speedup 1.0000x reference)
"""Marching-tetrahedra (BCCMarchingTetrahedraHelper) kernel for Trainium2,
SPMD across 8 NeuronCores.

Problem structure
-----------------
T = 2^20 tets; tet t owns vertices [4t, 4t+3] (edges_base = arange(T)*4), so
edges never collide across tets, and within a tet the 6 BASE_TET_EDGES rows
are already lexicographically sorted.  Hence the reference's
jnp.unique(sorted_edges, axis=0, return_inverse=True) is an identity
permutation (unique_edges == all_edges, idx_map == arange), and the
"distributed dedup" reduces to a cumsum over the dense crossing mask.
Invalid (non-surface) tets contribute zero crossing edges, so that cumsum
can be taken over ALL tets densely — no compaction needed before the scan.

Device (data-parallel over contiguous tet blocks, 1/8 per core)
---------------------------------------------------------------
Reads its sdf/pos shard and computes dense per-edge interpolated vertices
    v[t,e,:] = p_j + t_e * (p_i - p_j),   t_e = s_j / (s_j - s_i)
for all 6 edges of every tet (garbage in non-crossing lanes; host discards
them).  This is the memory/vector-bound bulk of the problem.

Performance notes (measured on TRN2):
  - Host pre-transposes each core's shard into per-partition SoA planes
    (sdf [v][t] f32, pos [v][c][t] fp16), so every DVE op runs on
    inner-stride-1 access patterns; fp16 tensor-tensor ops then hit the
    2x DVE perf mode (~0.51 cyc/el vs ~1.8 for element-strided f32).
  - den and t also run fp16; only the reciprocal itself is f32
    (reciprocal_approx_fast is an f32-only custom DVE op), so ACT
    upcasts den16 -> f32 and downcasts r -> fp16 around it.
  - GPSIMD offload was tried and abandoned: GPSIMD shares SBUF ports with
    the DVE, so concurrent tensor-tensor work on both engines just
    stretches both (~zero net gain).
  - 5 chunks (32/320/352/288/32 tets per partition) double-buffered: small
    head chunk starts compute early, small tail chunk shrinks the final
    store; ~46us DVE busy + ~19us fixed NEFF head/tail => ~66us.

Host
----
Integer topology (occupancy, crossing mask, cumsum vertex ids,
triangle-table lookups, stream compaction) in numpy — tiny vs. the
interpolation.  Faces come out bit-exact vs. the reference; verts carry
fp16 interpolation error (~3e-4 relative RMS, reference gate is 2e-2).
"""

import os as _os

import numpy as np

# ---------------------------------------------------------------- constants
T = 1 << 20            # tets
NCORES = 8
TC = T // NCORES       # tets per core        = 131072
P = 128                # SBUF partitions
L = TC // P            # tets per partition   = 1024

E_I = (0, 0, 0, 1, 1, 2)
E_J = (1, 2, 3, 2, 3, 3)
# edges grouped by shared base vertex i: (i, e_lo, e_hi); edge e in the
# group pairs (i, j = i + 1 + (e - e_lo)).
GROUPS = ((0, 0, 3), (1, 3, 5), (2, 5, 6))

_CHUNKS = tuple(int(x) for x in
                _os.environ.get("MT_CHUNKS", "32,320,352,288,32").split(","))
assert sum(_CHUNKS) == L

_TRIANGLE_TABLE = np.array([
    [-1, -1, -1, -1, -1, -1], [1, 0, 2, -1, -1, -1], [4, 0, 3, -1, -1, -1],
    [1, 4, 2, 1, 3, 4], [3, 1, 5, -1, -1, -1], [2, 3, 0, 2, 5, 3],
    [1, 4, 0, 1, 5, 4], [4, 2, 5, -1, -1, -1], [4, 5, 2, -1, -1, -1],
    [4, 1, 0, 4, 5, 1], [3, 2, 0, 3, 5, 2], [1, 3, 5, -1, -1, -1],
    [4, 1, 2, 4, 3, 1], [3, 0, 4, -1, -1, -1], [2, 0, 1, -1, -1, -1],
    [-1, -1, -1, -1, -1, -1]], dtype=np.int32)
_NUM_TRIANGLES = np.array([0, 1, 1, 2, 1, 2, 2, 1, 1, 2, 2, 1, 2, 1, 1, 0],
                          dtype=np.int32)

_COMPILED = {}


# ---------------------------------------------------------------- device
def _build_nc():
    import concourse.tile as tile
    from concourse import bacc, mybir
    from contextlib import ExitStack

    f32 = mybir.dt.float32
    f16 = mybir.dt.float16
    Cmax = max(_CHUNKS)
    nc = bacc.Bacc("TRN2", target_bir_lowering=False, debug=False)
    sdf = nc.dram_tensor("sdf", [P, 4 * L], f16, kind="ExternalInput")
    pos = nc.dram_tensor("pos", [P, 12 * L], f16, kind="ExternalInput")
    verts = nc.dram_tensor("verts", [P, L * 18], f16, kind="ExternalOutput")
    sdfv = sdf.ap().rearrange("p (v t) -> p v t", v=4)
    posv = pos.ap().rearrange("p (v c t) -> p v c t", v=4, c=3)

    with tile.TileContext(nc) as tc, ExitStack() as ctx:
        in_pool = ctx.enter_context(tc.tile_pool(name="inp", bufs=1))
        out_pool = ctx.enter_context(tc.tile_pool(name="outp", bufs=2))
        tmp_pool = ctx.enter_context(tc.tile_pool(name="tmpp", bufs=2))

        # the fp16 SoA shards are small (8KB + 24KB per partition): load
        # them whole up front — chunks are compute+store only, so no
        # chunk-boundary input waits at all
        st = in_pool.tile([P, 4 * L], f16, tag="sdf")
        sv_all = st[:].rearrange("p (v t) -> p v t", v=4)
        nc.sync.dma_start(sv_all, sdfv)
        pt = in_pool.tile([P, 12 * L], f16, tag="pos")
        pv_all = pt[:].rearrange("p (v c t) -> p v c t", v=4, c=3)
        split = _CHUNKS[0] + _CHUNKS[1]
        nc.sync.dma_start(pv_all[:, :, :, :split], posv[:, :, :, :split])
        nc.sync.dma_start(pv_all[:, :, :, split:], posv[:, :, :, split:])

        off = 0
        for C in _CHUNKS:
            sv = sv_all[:, :, off:off + C]
            pv = pv_all[:, :, :, off:off + C]
            vt = out_pool.tile([P, Cmax * 18], f16, tag="vout")
            vo = vt[:, :C * 18].rearrange("p (e c t) -> p e c t", e=6, c=3)

            # den16[e][t] = s_j - s_i  (fp16, 2x DVE mode); the f32-only
            # reciprocal runs on an ACT upcast, ACT casts r back to fp16
            den = tmp_pool.tile([P, Cmax * 6], f16, tag="den")
            d3 = den[:, :C * 6].rearrange("p (e t) -> p e t", e=6)
            for (i, elo, ehi) in GROUPS:
                n = ehi - elo
                sj = sv[:, i + 1:i + 1 + n, :]
                sib = sv[:, i:i + 1, :].to_broadcast((P, n, C))
                nc.vector.tensor_sub(d3[:, elo:ehi, :], sj, sib)
            d32 = tmp_pool.tile([P, Cmax * 6], f32, tag="d32")
            nc.scalar.copy(d32[:, :C * 6], den[:, :C * 6])
            r32 = tmp_pool.tile([P, Cmax * 6], f32, tag="r32")
            nc.vector.reciprocal_approx_fast(
                out=r32[:, :C * 6], in_=d32[:, :C * 6])
            r16 = tmp_pool.tile([P, Cmax * 6], f16, tag="r16")
            nc.scalar.copy(r16[:, :C * 6], r32[:, :C * 6])
            r3 = r16[:, :C * 6].rearrange("p (e t) -> p e t", e=6)
            t16 = tmp_pool.tile([P, Cmax * 6], f16, tag="t16")
            t3 = t16[:, :C * 6].rearrange("p (e t) -> p e t", e=6)
            for (i, elo, ehi) in GROUPS:
                n = ehi - elo
                sj = sv[:, i + 1:i + 1 + n, :]
                nc.vector.tensor_mul(t3[:, elo:ehi, :], sj, r3[:, elo:ehi, :])

            # delta = p_i - p_j ; v = t*delta + p_j    (fp16, 2x DVE mode)
            delta = tmp_pool.tile([P, Cmax * 18], f16, tag="delta")
            dl = delta[:, :C * 18].rearrange("p (e c t) -> p e c t", e=6, c=3)
            for (i, elo, ehi) in GROUPS:
                n = ehi - elo
                pj = pv[:, i + 1:i + 1 + n, :, :]
                pib = pv[:, i:i + 1, :, :].to_broadcast((P, n, 3, C))
                nc.vector.tensor_sub(dl[:, elo:ehi, :, :], pib, pj)
            tb = t3.unsqueeze(2).to_broadcast((P, 6, 3, C))
            nc.vector.tensor_mul(vo, tb, dl)
            for (i, elo, ehi) in GROUPS:
                n = ehi - elo
                pj = pv[:, i + 1:i + 1 + n, :, :]
                nc.vector.tensor_add(vo[:, elo:ehi, :, :],
                                     vo[:, elo:ehi, :, :], pj)
                # store each edge-group as soon as its add lands — spreads
                # the output DMA and shrinks the end-of-kernel drain
                nc.sync.dma_start(
                    verts.ap()[:, off * 18 + elo * 3 * C:
                               off * 18 + ehi * 3 * C],
                    vt[:, elo * 3 * C:ehi * 3 * C])
            off += C

    nc.compile()
    return nc


def _get_nc():
    if "nc" not in _COMPILED:
        _COMPILED["nc"] = _build_nc()
    return _COMPILED["nc"]


def _make_in_maps(sdf_n, pos_nx3):
    # per-partition SoA planes, fp16: sdf [P, v, t], pos [P, v*c, t]
    sdf_c = np.ascontiguousarray(
        np.asarray(sdf_n, np.float16).reshape(NCORES, P, L, 4)
        .transpose(0, 1, 3, 2)).reshape(NCORES, P, 4 * L)
    pos_c = np.ascontiguousarray(
        np.asarray(pos_nx3, np.float16).reshape(NCORES, P, L, 12)
        .transpose(0, 1, 3, 2)).reshape(NCORES, P, 12 * L)
    return [{"sdf": sdf_c[c], "pos": pos_c[c]} for c in range(NCORES)]


def _device_dense_verts(sdf_n, pos_nx3):
    """Run the SPMD kernel; returns dense verts [T, 6, 3] float32."""
    from concourse.bass_utils import run_bass_kernel_spmd

    nc = _get_nc()
    res = run_bass_kernel_spmd(nc, _make_in_maps(sdf_n, pos_nx3),
                               core_ids=list(range(NCORES)))
    # device layout per partition row: chunk blocks of [18, C] fp16 in
    # (e*3+c, t) order
    out = np.empty((T, 6, 3), dtype=np.float32)
    ov = out.reshape(NCORES, P, L, 18)
    raw = np.stack([res.results[c]["verts"] for c in range(NCORES)])
    off = 0
    for C in _CHUNKS:
        blk = raw[:, :, off * 18:(off + C) * 18].reshape(NCORES, P, 18, C)
        ov[:, :, off:off + C, :] = blk.transpose(0, 1, 3, 2)
        off += C
    return out


# ---------------------------------------------------------------- host
def _host_topology(occ4):
    cross = occ4[:, E_I] != occ4[:, E_J]                     # [T,6]
    flat = cross.reshape(-1)
    vid = np.cumsum(flat, dtype=np.int64) - 1
    mapping = np.where(flat, vid, -1).astype(np.int32).reshape(T, 6)
    tetindex = (occ4.astype(np.int32) *
                np.array([1, 2, 4, 8], np.int32)).sum(1)
    num_tri = _NUM_TRIANGLES[tetindex]
    tt = _TRIANGLE_TABLE[tetindex]
    return cross, mapping, num_tri == 1, num_tri == 2, tt


def kernel(sdf_n, pos_nx3, edges_base):
    sdf_n = np.asarray(sdf_n, dtype=np.float32)
    pos_nx3 = np.asarray(pos_nx3, dtype=np.float32)

    dense = _device_dense_verts(sdf_n, pos_nx3)              # [T,6,3]

    occ4 = (sdf_n > 0.0).reshape(T, 4)
    cross, mapping, m1, m2, tt = _host_topology(occ4)

    verts = dense.reshape(T * 6, 3)[cross.reshape(-1)]

    f1 = np.take_along_axis(mapping[m1], tt[m1][:, :3], axis=1).reshape(-1, 3)
    f2 = np.take_along_axis(mapping[m2], tt[m2][:, :6], axis=1).reshape(-1, 3)
    faces = np.concatenate([f1, f2], axis=0).astype(np.int32)
    return verts, faces


# revision 55
# speedup vs baseline: 1.0207x; 1.0207x over previous
"""Marching-tetrahedra (BCCMarchingTetrahedraHelper) kernel for Trainium2,
SPMD across 8 NeuronCores.

Problem structure
-----------------
T = 2^20 tets; tet t owns vertices [4t, 4t+3] (edges_base = arange(T)*4), so
edges never collide across tets, and within a tet the 6 BASE_TET_EDGES rows
are already lexicographically sorted.  Hence the reference's
jnp.unique(sorted_edges, axis=0, return_inverse=True) is an identity
permutation (unique_edges == all_edges, idx_map == arange), and the
"distributed dedup" reduces to a cumsum over the dense crossing mask.
Invalid (non-surface) tets contribute zero crossing edges, so that cumsum
can be taken over ALL tets densely — no compaction needed before the scan.

Device (data-parallel over contiguous tet blocks, 1/8 per core)
---------------------------------------------------------------
Reads its sdf/pos shard and computes dense per-edge interpolated vertices
    v[t,e,:] = p_j + t_e * (p_i - p_j),   t_e = s_j / (s_j - s_i)
for all 6 edges of every tet (garbage in non-crossing lanes; host discards
them).  This is the memory/vector-bound bulk of the problem.

Performance notes (measured on TRN2):
  - Host pre-transposes each core's shard into per-partition SoA planes
    (sdf [v][t] f32, pos [v][c][t] fp16), so every DVE op runs on
    inner-stride-1 access patterns; fp16 tensor-tensor ops then hit the
    2x DVE perf mode (~0.51 cyc/el vs ~1.8 for element-strided f32).
  - den and t also run fp16; only the reciprocal itself is f32
    (reciprocal_approx_fast is an f32-only custom DVE op), so ACT
    upcasts den16 -> f32 and downcasts r -> fp16 around it.
  - GPSIMD offload was tried and abandoned: GPSIMD shares SBUF ports with
    the DVE, so concurrent tensor-tensor work on both engines just
    stretches both (~zero net gain).
  - 5 chunks (32/320/352/288/32 tets per partition) double-buffered: small
    head chunk starts compute early, small tail chunk shrinks the final
    store; ~46us DVE busy + ~19us fixed NEFF head/tail => ~66us.

Host
----
Integer topology (occupancy, crossing mask, cumsum vertex ids,
triangle-table lookups, stream compaction) in numpy — tiny vs. the
interpolation.  Faces come out bit-exact vs. the reference; verts carry
fp16 interpolation error (~3e-4 relative RMS, reference gate is 2e-2).
"""

import os as _os

import numpy as np

# ---------------------------------------------------------------- constants
T = 1 << 20            # tets
NCORES = 8
TC = T // NCORES       # tets per core        = 131072
P = 128                # SBUF partitions
L = TC // P            # tets per partition   = 1024

E_I = (0, 0, 0, 1, 1, 2)
E_J = (1, 2, 3, 2, 3, 3)
# edges grouped by shared base vertex i: (i, e_lo, e_hi); edge e in the
# group pairs (i, j = i + 1 + (e - e_lo)).
GROUPS = ((0, 0, 3), (1, 3, 5), (2, 5, 6))

_CHUNKS = tuple(int(x) for x in
                _os.environ.get("MT_CHUNKS", "32,320,352,288,32").split(","))
assert sum(_CHUNKS) == L

_TRIANGLE_TABLE = np.array([
    [-1, -1, -1, -1, -1, -1], [1, 0, 2, -1, -1, -1], [4, 0, 3, -1, -1, -1],
    [1, 4, 2, 1, 3, 4], [3, 1, 5, -1, -1, -1], [2, 3, 0, 2, 5, 3],
    [1, 4, 0, 1, 5, 4], [4, 2, 5, -1, -1, -1], [4, 5, 2, -1, -1, -1],
    [4, 1, 0, 4, 5, 1], [3, 2, 0, 3, 5, 2], [1, 3, 5, -1, -1, -1],
    [4, 1, 2, 4, 3, 1], [3, 0, 4, -1, -1, -1], [2, 0, 1, -1, -1, -1],
    [-1, -1, -1, -1, -1, -1]], dtype=np.int32)
_NUM_TRIANGLES = np.array([0, 1, 1, 2, 1, 2, 2, 1, 1, 2, 2, 1, 2, 1, 1, 0],
                          dtype=np.int32)

_COMPILED = {}


# ---------------------------------------------------------------- device
def _build_nc():
    import concourse.tile as tile
    from concourse import bacc, mybir
    from contextlib import ExitStack

    f32 = mybir.dt.float32
    f16 = mybir.dt.float16
    Cmax = max(_CHUNKS)
    nc = bacc.Bacc("TRN2", target_bir_lowering=False, debug=False)
    sdf = nc.dram_tensor("sdf", [P, 4 * L], f16, kind="ExternalInput")
    pos = nc.dram_tensor("pos", [P, 12 * L], f16, kind="ExternalInput")
    verts = nc.dram_tensor("verts", [P, L * 18], f16, kind="ExternalOutput")
    sdfv = sdf.ap().rearrange("p (v t) -> p v t", v=4)
    posv = pos.ap().rearrange("p (v c t) -> p v c t", v=4, c=3)

    with tile.TileContext(nc) as tc, ExitStack() as ctx:
        in_pool = ctx.enter_context(tc.tile_pool(name="inp", bufs=1))
        out_pool = ctx.enter_context(tc.tile_pool(name="outp", bufs=2))
        tmp_pool = ctx.enter_context(tc.tile_pool(name="tmpp", bufs=2))

        # the fp16 SoA shards are small (8KB + 24KB per partition): load
        # them whole up front — chunks are compute+store only, so no
        # chunk-boundary input waits at all
        st = in_pool.tile([P, 4 * L], f16, tag="sdf")
        sv_all = st[:].rearrange("p (v t) -> p v t", v=4)
        nc.sync.dma_start(sv_all, sdfv)
        pt = in_pool.tile([P, 12 * L], f16, tag="pos")
        pv_all = pt[:].rearrange("p (v c t) -> p v c t", v=4, c=3)

        off = 0
        for C in _CHUNKS:
            sv = sv_all[:, :, off:off + C]
            pv = pv_all[:, :, :, off:off + C]
            nc.sync.dma_start(pv, posv[:, :, :, off:off + C])
            vt = out_pool.tile([P, Cmax * 18], f16, tag="vout")
            vo = vt[:, :C * 18].rearrange("p (e c t) -> p e c t", e=6, c=3)

            # den16[e][t] = s_j - s_i  (fp16, 2x DVE mode); the f32-only
            # reciprocal runs on an ACT upcast, ACT casts r back to fp16
            den = tmp_pool.tile([P, Cmax * 6], f16, tag="den")
            d3 = den[:, :C * 6].rearrange("p (e t) -> p e t", e=6)
            for (i, elo, ehi) in GROUPS:
                n = ehi - elo
                sj = sv[:, i + 1:i + 1 + n, :]
                sib = sv[:, i:i + 1, :].to_broadcast((P, n, C))
                nc.vector.tensor_sub(d3[:, elo:ehi, :], sj, sib)
            d32 = tmp_pool.tile([P, Cmax * 6], f32, tag="d32")
            nc.scalar.copy(d32[:, :C * 6], den[:, :C * 6])
            r32 = tmp_pool.tile([P, Cmax * 6], f32, tag="r32")
            nc.vector.reciprocal_approx_fast(
                out=r32[:, :C * 6], in_=d32[:, :C * 6])
            r16 = tmp_pool.tile([P, Cmax * 6], f16, tag="r16")
            nc.scalar.copy(r16[:, :C * 6], r32[:, :C * 6])
            r3 = r16[:, :C * 6].rearrange("p (e t) -> p e t", e=6)
            t16 = tmp_pool.tile([P, Cmax * 6], f16, tag="t16")
            t3 = t16[:, :C * 6].rearrange("p (e t) -> p e t", e=6)
            for (i, elo, ehi) in GROUPS:
                n = ehi - elo
                sj = sv[:, i + 1:i + 1 + n, :]
                nc.vector.tensor_mul(t3[:, elo:ehi, :], sj, r3[:, elo:ehi, :])

            # delta = p_i - p_j ; v = t*delta + p_j    (fp16, 2x DVE mode)
            delta = tmp_pool.tile([P, Cmax * 18], f16, tag="delta")
            dl = delta[:, :C * 18].rearrange("p (e c t) -> p e c t", e=6, c=3)
            for (i, elo, ehi) in GROUPS:
                n = ehi - elo
                pj = pv[:, i + 1:i + 1 + n, :, :]
                pib = pv[:, i:i + 1, :, :].to_broadcast((P, n, 3, C))
                nc.vector.tensor_sub(dl[:, elo:ehi, :, :], pib, pj)
            tb = t3.unsqueeze(2).to_broadcast((P, 6, 3, C))
            nc.vector.tensor_mul(vo, tb, dl)
            for (i, elo, ehi) in GROUPS:
                n = ehi - elo
                pj = pv[:, i + 1:i + 1 + n, :, :]
                nc.vector.tensor_add(vo[:, elo:ehi, :, :],
                                     vo[:, elo:ehi, :, :], pj)
                # store each edge-group as soon as its add lands — spreads
                # the output DMA and shrinks the end-of-kernel drain
                nc.sync.dma_start(
                    verts.ap()[:, off * 18 + elo * 3 * C:
                               off * 18 + ehi * 3 * C],
                    vt[:, elo * 3 * C:ehi * 3 * C])
            off += C

    nc.compile()
    return nc


def _get_nc():
    if "nc" not in _COMPILED:
        _COMPILED["nc"] = _build_nc()
    return _COMPILED["nc"]


def _make_in_maps(sdf_n, pos_nx3):
    # per-partition SoA planes, fp16: sdf [P, v, t], pos [P, v*c, t]
    sdf_c = np.ascontiguousarray(
        np.asarray(sdf_n, np.float16).reshape(NCORES, P, L, 4)
        .transpose(0, 1, 3, 2)).reshape(NCORES, P, 4 * L)
    pos_c = np.ascontiguousarray(
        np.asarray(pos_nx3, np.float16).reshape(NCORES, P, L, 12)
        .transpose(0, 1, 3, 2)).reshape(NCORES, P, 12 * L)
    return [{"sdf": sdf_c[c], "pos": pos_c[c]} for c in range(NCORES)]


def _device_dense_verts(sdf_n, pos_nx3):
    """Run the SPMD kernel; returns dense verts [T, 6, 3] float32."""
    from concourse.bass_utils import run_bass_kernel_spmd

    nc = _get_nc()
    res = run_bass_kernel_spmd(nc, _make_in_maps(sdf_n, pos_nx3),
                               core_ids=list(range(NCORES)))
    # device layout per partition row: chunk blocks of [18, C] fp16 in
    # (e*3+c, t) order
    out = np.empty((T, 6, 3), dtype=np.float32)
    ov = out.reshape(NCORES, P, L, 18)
    raw = np.stack([res.results[c]["verts"] for c in range(NCORES)])
    off = 0
    for C in _CHUNKS:
        blk = raw[:, :, off * 18:(off + C) * 18].reshape(NCORES, P, 18, C)
        ov[:, :, off:off + C, :] = blk.transpose(0, 1, 3, 2)
        off += C
    return out


# ---------------------------------------------------------------- host
def _host_topology(occ4):
    cross = occ4[:, E_I] != occ4[:, E_J]                     # [T,6]
    flat = cross.reshape(-1)
    vid = np.cumsum(flat, dtype=np.int64) - 1
    mapping = np.where(flat, vid, -1).astype(np.int32).reshape(T, 6)
    tetindex = (occ4.astype(np.int32) *
                np.array([1, 2, 4, 8], np.int32)).sum(1)
    num_tri = _NUM_TRIANGLES[tetindex]
    tt = _TRIANGLE_TABLE[tetindex]
    return cross, mapping, num_tri == 1, num_tri == 2, tt


def kernel(sdf_n, pos_nx3, edges_base):
    sdf_n = np.asarray(sdf_n, dtype=np.float32)
    pos_nx3 = np.asarray(pos_nx3, dtype=np.float32)

    dense = _device_dense_verts(sdf_n, pos_nx3)              # [T,6,3]

    occ4 = (sdf_n > 0.0).reshape(T, 4)
    cross, mapping, m1, m2, tt = _host_topology(occ4)

    verts = dense.reshape(T * 6, 3)[cross.reshape(-1)]

    f1 = np.take_along_axis(mapping[m1], tt[m1][:, :3], axis=1).reshape(-1, 3)
    f2 = np.take_along_axis(mapping[m2], tt[m2][:, :6], axis=1).reshape(-1, 3)
    faces = np.concatenate([f1, f2], axis=0).astype(np.int32)
    return verts, faces


# revision 56
# speedup vs baseline: 1.0509x; 1.0296x over previous
"""Marching-tetrahedra (BCCMarchingTetrahedraHelper) kernel for Trainium2,
SPMD across 8 NeuronCores.

Problem structure
-----------------
T = 2^20 tets; tet t owns vertices [4t, 4t+3] (edges_base = arange(T)*4), so
edges never collide across tets, and within a tet the 6 BASE_TET_EDGES rows
are already lexicographically sorted.  Hence the reference's
jnp.unique(sorted_edges, axis=0, return_inverse=True) is an identity
permutation (unique_edges == all_edges, idx_map == arange), and the
"distributed dedup" reduces to a cumsum over the dense crossing mask.
Invalid (non-surface) tets contribute zero crossing edges, so that cumsum
can be taken over ALL tets densely — no compaction needed before the scan.

Device (data-parallel over contiguous tet blocks, 1/8 per core)
---------------------------------------------------------------
Reads its sdf/pos shard and computes dense per-edge interpolated vertices
    v[t,e,:] = p_j + t_e * (p_i - p_j),   t_e = s_j / (s_j - s_i)
for all 6 edges of every tet (garbage in non-crossing lanes; host discards
them).  This is the memory/vector-bound bulk of the problem.

Performance notes (measured on TRN2):
  - Host pre-transposes each core's shard into per-partition SoA planes
    (sdf [v][t] f32, pos [v][c][t] fp16), so every DVE op runs on
    inner-stride-1 access patterns; fp16 tensor-tensor ops then hit the
    2x DVE perf mode (~0.51 cyc/el vs ~1.8 for element-strided f32).
  - den and t also run fp16; only the reciprocal itself is f32
    (reciprocal_approx_fast is an f32-only custom DVE op), so ACT
    upcasts den16 -> f32 and downcasts r -> fp16 around it.
  - GPSIMD offload was tried and abandoned: GPSIMD shares SBUF ports with
    the DVE, so concurrent tensor-tensor work on both engines just
    stretches both (~zero net gain).
  - 5 chunks (32/320/352/288/32 tets per partition) double-buffered: small
    head chunk starts compute early, small tail chunk shrinks the final
    store; ~46us DVE busy + ~19us fixed NEFF head/tail => ~66us.

Host
----
Integer topology (occupancy, crossing mask, cumsum vertex ids,
triangle-table lookups, stream compaction) in numpy — tiny vs. the
interpolation.  Faces come out bit-exact vs. the reference; verts carry
fp16 interpolation error (~3e-4 relative RMS, reference gate is 2e-2).
"""

import os as _os

import numpy as np

# ---------------------------------------------------------------- constants
T = 1 << 20            # tets
NCORES = 8
TC = T // NCORES       # tets per core        = 131072
P = 128                # SBUF partitions
L = TC // P            # tets per partition   = 1024

E_I = (0, 0, 0, 1, 1, 2)
E_J = (1, 2, 3, 2, 3, 3)
# edges grouped by shared base vertex i: (i, e_lo, e_hi); edge e in the
# group pairs (i, j = i + 1 + (e - e_lo)).
GROUPS = ((0, 0, 3), (1, 3, 5), (2, 5, 6))

_CHUNKS = tuple(int(x) for x in
                _os.environ.get("MT_CHUNKS", "32,320,352,288,32").split(","))
assert sum(_CHUNKS) == L

_TRIANGLE_TABLE = np.array([
    [-1, -1, -1, -1, -1, -1], [1, 0, 2, -1, -1, -1], [4, 0, 3, -1, -1, -1],
    [1, 4, 2, 1, 3, 4], [3, 1, 5, -1, -1, -1], [2, 3, 0, 2, 5, 3],
    [1, 4, 0, 1, 5, 4], [4, 2, 5, -1, -1, -1], [4, 5, 2, -1, -1, -1],
    [4, 1, 0, 4, 5, 1], [3, 2, 0, 3, 5, 2], [1, 3, 5, -1, -1, -1],
    [4, 1, 2, 4, 3, 1], [3, 0, 4, -1, -1, -1], [2, 0, 1, -1, -1, -1],
    [-1, -1, -1, -1, -1, -1]], dtype=np.int32)
_NUM_TRIANGLES = np.array([0, 1, 1, 2, 1, 2, 2, 1, 1, 2, 2, 1, 2, 1, 1, 0],
                          dtype=np.int32)

_COMPILED = {}


# ---------------------------------------------------------------- device
def _build_nc():
    import concourse.tile as tile
    from concourse import bacc, mybir
    from contextlib import ExitStack

    f32 = mybir.dt.float32
    f16 = mybir.dt.float16
    Cmax = max(_CHUNKS)
    nc = bacc.Bacc("TRN2", target_bir_lowering=False, debug=False)
    sdf = nc.dram_tensor("sdf", [P, 4 * L], f16, kind="ExternalInput")
    pos = nc.dram_tensor("pos", [P, 12 * L], f16, kind="ExternalInput")
    verts = nc.dram_tensor("verts", [P, L * 18], f16, kind="ExternalOutput")
    sdfv = sdf.ap().rearrange("p (v t) -> p v t", v=4)
    posv = pos.ap().rearrange("p (v c t) -> p v c t", v=4, c=3)

    with tile.TileContext(nc) as tc, ExitStack() as ctx:
        in_pool = ctx.enter_context(tc.tile_pool(name="inp", bufs=2))
        out_pool = ctx.enter_context(tc.tile_pool(name="outp", bufs=2))
        tmp_pool = ctx.enter_context(tc.tile_pool(name="tmpp", bufs=2))

        off = 0
        for C in _CHUNKS:
            st = in_pool.tile([P, Cmax * 4], f16, tag="sdf")
            sv = st[:, :C * 4].rearrange("p (v t) -> p v t", v=4)
            nc.sync.dma_start(sv, sdfv[:, :, off:off + C])
            pt = in_pool.tile([P, Cmax * 12], f16, tag="pos")
            pv = pt[:, :C * 12].rearrange("p (v c t) -> p v c t", v=4, c=3)
            nc.sync.dma_start(pv, posv[:, :, :, off:off + C])
            vt = out_pool.tile([P, Cmax * 18], f16, tag="vout")
            vo = vt[:, :C * 18].rearrange("p (e c t) -> p e c t", e=6, c=3)

            # den16[e][t] = s_j - s_i  (fp16, 2x DVE mode); the f32-only
            # reciprocal runs on an ACT upcast, ACT casts r back to fp16
            den = tmp_pool.tile([P, Cmax * 6], f16, tag="den")
            d3 = den[:, :C * 6].rearrange("p (e t) -> p e t", e=6)
            for (i, elo, ehi) in GROUPS:
                n = ehi - elo
                sj = sv[:, i + 1:i + 1 + n, :]
                sib = sv[:, i:i + 1, :].to_broadcast((P, n, C))
                nc.vector.tensor_sub(d3[:, elo:ehi, :], sj, sib)
            d32 = tmp_pool.tile([P, Cmax * 6], f32, tag="d32")
            nc.scalar.copy(d32[:, :C * 6], den[:, :C * 6])
            r32 = tmp_pool.tile([P, Cmax * 6], f32, tag="r32")
            nc.vector.reciprocal_approx_fast(
                out=r32[:, :C * 6], in_=d32[:, :C * 6])
            r16 = tmp_pool.tile([P, Cmax * 6], f16, tag="r16")
            nc.scalar.copy(r16[:, :C * 6], r32[:, :C * 6])
            r3 = r16[:, :C * 6].rearrange("p (e t) -> p e t", e=6)
            t16 = tmp_pool.tile([P, Cmax * 6], f16, tag="t16")
            t3 = t16[:, :C * 6].rearrange("p (e t) -> p e t", e=6)
            for (i, elo, ehi) in GROUPS:
                n = ehi - elo
                sj = sv[:, i + 1:i + 1 + n, :]
                nc.vector.tensor_mul(t3[:, elo:ehi, :], sj, r3[:, elo:ehi, :])

            # delta = p_i - p_j ; v = t*delta + p_j    (fp16, 2x DVE mode)
            delta = tmp_pool.tile([P, Cmax * 18], f16, tag="delta")
            dl = delta[:, :C * 18].rearrange("p (e c t) -> p e c t", e=6, c=3)
            for (i, elo, ehi) in GROUPS:
                n = ehi - elo
                pj = pv[:, i + 1:i + 1 + n, :, :]
                pib = pv[:, i:i + 1, :, :].to_broadcast((P, n, 3, C))
                nc.vector.tensor_sub(dl[:, elo:ehi, :, :], pib, pj)
            tb = t3.unsqueeze(2).to_broadcast((P, 6, 3, C))
            nc.vector.tensor_mul(vo, tb, dl)
            for (i, elo, ehi) in GROUPS:
                n = ehi - elo
                pj = pv[:, i + 1:i + 1 + n, :, :]
                nc.vector.tensor_add(vo[:, elo:ehi, :, :],
                                     vo[:, elo:ehi, :, :], pj)
                # store each edge-group as soon as its add lands — spreads
                # the output DMA and shrinks the end-of-kernel drain
                nc.sync.dma_start(
                    verts.ap()[:, off * 18 + elo * 3 * C:
                               off * 18 + ehi * 3 * C],
                    vt[:, elo * 3 * C:ehi * 3 * C])
            off += C

    nc.compile()
    return nc


def _get_nc():
    if "nc" not in _COMPILED:
        _COMPILED["nc"] = _build_nc()
    return _COMPILED["nc"]


def _make_in_maps(sdf_n, pos_nx3):
    # per-partition SoA planes, fp16: sdf [P, v, t], pos [P, v*c, t]
    sdf_c = np.ascontiguousarray(
        np.asarray(sdf_n, np.float16).reshape(NCORES, P, L, 4)
        .transpose(0, 1, 3, 2)).reshape(NCORES, P, 4 * L)
    pos_c = np.ascontiguousarray(
        np.asarray(pos_nx3, np.float16).reshape(NCORES, P, L, 12)
        .transpose(0, 1, 3, 2)).reshape(NCORES, P, 12 * L)
    return [{"sdf": sdf_c[c], "pos": pos_c[c]} for c in range(NCORES)]


def _device_dense_verts(sdf_n, pos_nx3):
    """Run the SPMD kernel; returns dense verts [T, 6, 3] float32."""
    from concourse.bass_utils import run_bass_kernel_spmd

    nc = _get_nc()
    res = run_bass_kernel_spmd(nc, _make_in_maps(sdf_n, pos_nx3),
                               core_ids=list(range(NCORES)))
    # device layout per partition row: chunk blocks of [18, C] fp16 in
    # (e*3+c, t) order
    out = np.empty((T, 6, 3), dtype=np.float32)
    ov = out.reshape(NCORES, P, L, 18)
    raw = np.stack([res.results[c]["verts"] for c in range(NCORES)])
    off = 0
    for C in _CHUNKS:
        blk = raw[:, :, off * 18:(off + C) * 18].reshape(NCORES, P, 18, C)
        ov[:, :, off:off + C, :] = blk.transpose(0, 1, 3, 2)
        off += C
    return out


# ---------------------------------------------------------------- host
def _host_topology(occ4):
    cross = occ4[:, E_I] != occ4[:, E_J]                     # [T,6]
    flat = cross.reshape(-1)
    vid = np.cumsum(flat, dtype=np.int64) - 1
    mapping = np.where(flat, vid, -1).astype(np.int32).reshape(T, 6)
    tetindex = (occ4.astype(np.int32) *
                np.array([1, 2, 4, 8], np.int32)).sum(1)
    num_tri = _NUM_TRIANGLES[tetindex]
    tt = _TRIANGLE_TABLE[tetindex]
    return cross, mapping, num_tri == 1, num_tri == 2, tt


def kernel(sdf_n, pos_nx3, edges_base):
    sdf_n = np.asarray(sdf_n, dtype=np.float32)
    pos_nx3 = np.asarray(pos_nx3, dtype=np.float32)

    dense = _device_dense_verts(sdf_n, pos_nx3)              # [T,6,3]

    occ4 = (sdf_n > 0.0).reshape(T, 4)
    cross, mapping, m1, m2, tt = _host_topology(occ4)

    verts = dense.reshape(T * 6, 3)[cross.reshape(-1)]

    f1 = np.take_along_axis(mapping[m1], tt[m1][:, :3], axis=1).reshape(-1, 3)
    f2 = np.take_along_axis(mapping[m2], tt[m2][:, :6], axis=1).reshape(-1, 3)
    faces = np.concatenate([f1, f2], axis=0).astype(np.int32)
    return verts, faces
